# revision 1
# baseline (speedup 1.0000x reference)
"""LIF spike kernel for Trainium2 (Bass/Tile), data-parallel over 8 NeuronCores.

Problem: x [32, 8, 128, 32, 32] fp32 -> spikes [32, 8, 128, 32, 32] fp32
    mem_t = mem_{t-1} * 0.25 + x_t ; spike = (mem >= 0.5) ; mem *= (1 - spike)

Sharding: batch dim (32) split 4-per-core across 8 cores; no cross-core comm.

Per-core device program (shapes [4, 8, 128, 1024]):
  - layout: partitions = channel dim C=128, free = (b, h*w) = 4096
  - per time step on VectorE:
        u   = (r * TAU) + x_t            scalar_tensor_tensor, fp32
        y_t = (u >= 0.5)                 tensor_scalar -> uint8 {0,1}
        r   = (u < 0.5) * u              scalar_tensor_tensor (reset)
  - spike output is uint8; host casts back to fp32 (exact, spikes are 0/1).
All arithmetic is fp32 and rounds identically to the jax reference
(mult by 0.25 is exact; a single rounding add per step), so the spike
train is expected to match bitwise.
"""

import os
import numpy as np

B, T, C, H, W = 32, 8, 128, 32, 32
HW = H * W
N_CORES = 8
BPC = B // N_CORES  # batches per core
TAU = 0.25
THRESH = 0.5

_nc_cache = {}
LAST_RESULTS = None


def build_bass(free_w=HW, use_act=False, reps=1):
    """Build the per-core Bass program. free_w lets tests shrink the spatial
    dim for fast simulation; reps>1 repeats the whole computation for
    loop-delta hardware timing (outputs are rewritten identically)."""
    import concourse.bacc as bacc
    import concourse.mybir as mybir
    from concourse.tile import TileContext

    f32 = mybir.dt.float32
    u8 = mybir.dt.uint8
    Alu = mybir.AluOpType

    nc = bacc.Bacc("TRN2", target_bir_lowering=False)
    x = nc.dram_tensor("x", [BPC, T, C, free_w], f32, kind="ExternalInput")
    y = nc.dram_tensor("y", [BPC, T, C, free_w], u8, kind="ExternalOutput")

    with TileContext(nc) as tc:
        with (
            tc.tile_pool(name="xp", bufs=6) as xp,
            tc.tile_pool(name="up", bufs=2) as up,
            tc.tile_pool(name="rp", bufs=2) as rp,
            tc.tile_pool(name="yp", bufs=3) as yp,
            tc.tile_pool(name="cp", bufs=1) as cp,
        ):
            neg_thresh = None
            if use_act:
                neg_thresh = cp.tile([C, 1], f32)
                nc.vector.memset(neg_thresh[:], -THRESH)
            for _rep in range(reps):
                r = None
                for t in range(T):
                    xt = xp.tile([C, BPC, free_w], f32)
                    nc.sync.dma_start(xt[:], x[:, t, :, :].rearrange("b c w -> c b w"))
                    if t == 0:
                        u = xt
                    else:
                        u = up.tile([C, BPC, free_w], f32)
                        nc.vector.scalar_tensor_tensor(
                            u[:], r[:], TAU, xt[:], Alu.mult, Alu.add
                        )
                    yt = yp.tile([C, BPC, free_w], u8)
                    if use_act:
                        # spike = Sign(u - 0.5) saturated to uint8: {-1,0,+1}->{0,0,1}
                        nc.scalar.activation(
                            yt[:],
                            u[:],
                            mybir.ActivationFunctionType.Sign,
                            bias=neg_thresh[:],
                        )
                    else:
                        nc.vector.tensor_scalar(yt[:], u[:], THRESH, None, Alu.is_ge)
                    if t < T - 1:
                        rn = rp.tile([C, BPC, free_w], f32)
                        nc.vector.scalar_tensor_tensor(
                            rn[:], u[:], THRESH, u[:], Alu.is_lt, Alu.mult
                        )
                        r = rn
                    # out-DMAs ride the second HWDGE ring (ACT) so they don't
                    # serialize behind the x loads on the SP ring
                    nc.scalar.dma_start(
                        y[:, t, :, :].rearrange("b c w -> c b w"), yt[:]
                    )
    nc.compile()
    return nc


def build_bass_pe(free_w=HW, reps=1, h_dt="float8e4", chunk=2048):
    """PE variant: per step t>=1, u = 0.125*I @ d + I @ x accumulated in PSUM
    (two diagonal fp32 matmuls per 512-col bank); ACT computes
    h = Sign(0.5 - u) in {+1,0,-1} (doubles as the spike output: spike iff
    h <= 0); DVE computes d = (h + 1) * u = 2*u*[u<0.5] in one fused op.
    The 2x in d is folded into the 0.125 weight (0.25/2)."""
    import concourse.bacc as bacc
    import concourse.mybir as mybir
    from concourse.tile import TileContext

    f32 = mybir.dt.float32
    Alu = mybir.AluOpType
    hdt = getattr(mybir.dt, h_dt)

    nc = bacc.Bacc("TRN2", target_bir_lowering=False)
    x = nc.dram_tensor("x", [BPC, T, C, free_w], f32, kind="ExternalInput")
    y = nc.dram_tensor("y", [BPC, T, C, free_w], hdt, kind="ExternalOutput")
    w8_d = nc.inline_tensor((np.eye(C) * (TAU / 2.0)).astype(np.float32), "w8")
    wid_d = nc.inline_tensor(np.eye(C, dtype=np.float32), "wid")

    FREE = BPC * free_w
    NCH = max(1, FREE // chunk)
    CH = FREE // NCH

    with TileContext(nc) as tc:
        with (
            tc.tile_pool(name="xp", bufs=3) as xp,
            tc.tile_pool(name="dp", bufs=2) as dp,
            tc.tile_pool(name="hp", bufs=3) as hp,
            tc.tile_pool(name="wp", bufs=1) as wp,
            tc.tile_pool(name="cp", bufs=1) as cp,
            tc.tile_pool(name="ps", bufs=2, space="PSUM") as ps,
        ):
            w8 = wp.tile([C, C], f32, tag="w8")
            wid = wp.tile([C, C], f32, tag="wid")
            nc.sync.dma_start(w8[:], w8_d[:])
            nc.sync.dma_start(wid[:], wid_d[:])
            half = cp.tile([C, 1], f32)
            nc.vector.memset(half[:], THRESH)
            for _rep in range(reps):
                d_prev = None
                for t in range(T):
                    xt = xp.tile([C, FREE], f32)
                    nc.sync.dma_start(
                        xt[:].rearrange("c (b w) -> c b w", b=BPC),
                        x[:, t, :, :].rearrange("b c w -> c b w"),
                    )
                    ht = hp.tile([C, FREE], hdt)
                    if t == 0:
                        # u_0 = x_0 lives in SBUF
                        nc.scalar.activation(
                            ht[:], xt[:], mybir.ActivationFunctionType.Sign,
                            bias=half[:], scale=-1.0,
                        )
                        if t < T - 1:
                            dn = dp.tile([C, FREE], f32, tag="d")
                            nc.vector.scalar_tensor_tensor(
                                dn[:], ht[:], 1.0, xt[:], Alu.add, Alu.mult
                            )
                            d_prev = dn
                    else:
                        if t < T - 1:
                            dn = dp.tile([C, FREE], f32, tag="d")
                        else:
                            dn = None
                        for j in range(NCH):
                            sl = slice(j * CH, (j + 1) * CH)
                            pt = ps.tile([C, CH], f32)
                            # matmul output is capped at one PSUM bank
                            # (512 fp32) — slice the psum tile bank-aligned
                            mmw = min(512, CH)
                            for k in range(0, CH, mmw):
                                kk = slice(k, k + mmw)
                                gsl = slice(j * CH + k, j * CH + k + mmw)
                                nc.tensor.matmul(
                                    pt[:, kk], w8[:], d_prev[:, gsl],
                                    start=True, stop=False,
                                )
                                nc.tensor.matmul(
                                    pt[:, kk], wid[:], xt[:, gsl],
                                    start=False, stop=True,
                                )
                            nc.scalar.activation(
                                ht[:, sl], pt[:],
                                mybir.ActivationFunctionType.Sign,
                                bias=half[:], scale=-1.0,
                            )
                            if dn is not None:
                                nc.vector.scalar_tensor_tensor(
                                    dn[:, sl], ht[:, sl], 1.0, pt[:],
                                    Alu.add, Alu.mult,
                                )
                        d_prev = dn
                    # second HWDGE ring (ACT) for stores, SP ring for loads
                    nc.scalar.dma_start(
                        y[:, t, :, :].rearrange("b c w -> c b w"),
                        ht[:].rearrange("c (b w) -> c b w", b=BPC),
                    )
    nc.compile()
    return nc


def _get_nc():
    variant = os.environ.get("LIF_VARIANT", "act")
    key = (HW, variant)
    if key not in _nc_cache:
        if variant == "pe":
            _nc_cache[key] = build_bass_pe(HW)
        else:
            _nc_cache[key] = build_bass(HW, use_act=variant == "act")
    return _nc_cache[key]


def kernel(x):
    global LAST_RESULTS
    from concourse import bass_utils

    assert x.shape == (B, T, C, H, W) and x.dtype == np.float32
    xs = np.ascontiguousarray(x.reshape(B, T, C, HW))
    nc = _get_nc()
    in_maps = [
        {"x": np.ascontiguousarray(xs[i * BPC : (i + 1) * BPC])}
        for i in range(N_CORES)
    ]
    res = bass_utils.run_bass_kernel_spmd(
        nc,
        in_maps,
        core_ids=list(range(N_CORES)),
        trace=bool(int(os.environ.get("LIF_TRACE", "0"))),
    )
    LAST_RESULTS = res
    variant = os.environ.get("LIF_VARIANT", "act")
    out = np.empty((B, T, C, HW), dtype=np.float32)
    for i in range(N_CORES):
        yi = res.results[i]["y"]
        if variant == "pe":
            # h = Sign(0.5-u) in fp8: +1 -> no spike; 0/-1 -> spike
            out[i * BPC : (i + 1) * BPC] = yi.astype(np.float32) < 0.5
        else:
            # spike iff raw uint8 == 1 (DVE is_ge gives {0,1}; ACT Sign gives
            # {-1,0,+1} which lands as {255/0, 0, 1} in uint8 depending on
            # wrap-vs-saturate — spike==1 holds in every case).
            out[i * BPC : (i + 1) * BPC] = yi == 1
    return out.reshape(B, T, C, H, W)



# revision 3
# speedup vs baseline: 1.3405x; 1.3405x over previous
"""LIF spike kernel for Trainium2 (Bass/Tile), data-parallel over 8 NeuronCores.

Problem: x [32, 8, 128, 32, 32] fp32 -> spikes [32, 8, 128, 32, 32] fp32
    mem_t = mem_{t-1} * 0.25 + x_t ; spike = (mem >= 0.5) ; mem *= (1 - spike)

Sharding: batch dim (32) split 4-per-core across 8 cores; no cross-core comm.

Per-core device program (host pre-transposes the core's slab to [T, C, B*HW]
so every DMA is a fully contiguous HBM stream; host undoes it after):
  - layout: partitions = channel dim C=128, free = (b, h*w) = 4096
  - membrane update is ONE fused custom DVE op per step (vs 2 stock ops):
        u_t = select(u_{t-1} < 0.5, u_{t-1}, 0) * TAU + x_t
    (mask-mult and *0.25 are exact in fp32; single rounding on the add —
    bitwise identical to the jax fp32 reference)
  - spike on the ACT engine: y_t = Sign(u_t - 0.5) -> uint8 {255/0, 0, 1};
    host decodes spike := (y == 1). ACT table is prewarmed before the loop.
  - x loads batched (STEPS_PER_LOAD steps per dma_start) on the SP HWDGE
    ring; y stores ride the ACT ring right after their Sign (program order,
    no extra sem wait).
"""

import os
import numpy as np

B, T, C, H, W = 32, 8, 128, 32, 32
HW = H * W
N_CORES = 8
BPC = B // N_CORES  # batches per core
FREE = BPC * HW  # 4096
TAU = 0.25
THRESH = 0.5

_nc_cache = {}
LAST_RESULTS = None


def _register_lif_op():
    """Register the fused LIF membrane-update op with the custom-DVE table
    (runtime equivalent of the documented two-edit dve_ops.py append)."""
    import concourse.dve_ops as dv
    from concourse.dve_spec import Spec, Src0, Src1, C0, C1, Zero, select

    for op in dv.OPS:
        if op.name == "LIF_FUSED_ANT":
            return op
    op = dv.DveOp(
        "LIF_FUSED_ANT",
        Spec(
            body=select(Src0 < C0, Src0, Zero) * C1 + Src1,
            reference=lambda in0, in1, s0, s1, imm2: (
                np.where(in0 < s0, in0, np.float32(0.0)) * np.float32(s1) + in1
            ).astype(np.float32),
        ),
        subdim=False,
        uops_sha={"v3": "dc49afe33bac4c9a", "v4": "05a48bcb07e07a04"},
    )
    dv.OPS.append(op)
    dv._SUB_OPCODE_FOR_NAME[op.name] = dv._CUSTOM_DVE_ROW_BASE + len(dv.OPS) - 1
    assert max(dv._SUB_OPCODE_FOR_NAME.values()) < 0x20
    return op


def build_bass_fused(reps=1, steps_per_load=2, nsplit=2, xp_bufs=None):
    """Per-core program on the [T, C, FREE] layout.

    steps_per_load: time steps batched into one input dma_start (bigger =
    better DMA efficiency, coarser pipeline granularity).
    nsplit: free-dim chunks per step for DVE/ACT (finer = shorter one-shot
    tail; throughput cost is the ~58-cycle per-op overhead).
    """
    import concourse.bacc as bacc
    import concourse.mybir as mybir
    from concourse.tile import TileContext

    lif_op = _register_lif_op()

    f32 = mybir.dt.float32
    u8 = mybir.dt.uint8

    assert T % steps_per_load == 0
    n_groups = T // steps_per_load
    if xp_bufs is None:
        # keep the whole-rep input (128 KiB/partition) outstanding
        xp_bufs = max(2, n_groups)
    assert FREE % nsplit == 0
    CH = FREE // nsplit

    nc = bacc.Bacc("TRN2", target_bir_lowering=False)
    x = nc.dram_tensor("x", [T, C, FREE], f32, kind="ExternalInput")
    y = nc.dram_tensor("y", [T, C, FREE], u8, kind="ExternalOutput")

    with TileContext(nc) as tc:
        with (
            tc.tile_pool(name="xp", bufs=xp_bufs) as xp,
            tc.tile_pool(name="up", bufs=3) as up,
            tc.tile_pool(name="yp", bufs=3) as yp,
            tc.tile_pool(name="cp", bufs=1) as cp,
        ):
            neg_thresh = cp.tile([C, 1], f32)
            nc.vector.memset(neg_thresh[:], -THRESH)
            warm = cp.tile([C, 1], u8)
            # prewarm the Sign table so the ~2.7us ACT_TABLE_LOAD overlaps
            # the first x load instead of sitting on the critical path
            nc.scalar.activation(
                warm[:], neg_thresh[:], mybir.ActivationFunctionType.Sign
            )
            for _rep in range(reps):
                xts = []
                for g in range(n_groups):
                    xt = xp.tile([C, steps_per_load, FREE], f32, tag="xt")
                    nc.sync.dma_start(
                        xt[:],
                        x[
                            g * steps_per_load : (g + 1) * steps_per_load
                        ].rearrange("t c w -> c t w"),
                    )
                    xts.append(xt)
                u_prev = None
                for t in range(T):
                    g, o = divmod(t, steps_per_load)
                    xsl = xts[g][:, o, :]
                    if t == 0:
                        u = xsl
                    else:
                        ut = up.tile([C, FREE], f32)
                        for j in range(nsplit):
                            s = slice(j * CH, (j + 1) * CH)
                            nc.vector._custom_dve(
                                lif_op,
                                out=ut[:, s],
                                in0=u_prev[:, s],
                                in1=xsl[:, s],
                                s0=THRESH,
                                s1=TAU,
                            )
                        u = ut
                    yt = yp.tile([C, FREE], u8)
                    for j in range(nsplit):
                        s = slice(j * CH, (j + 1) * CH)
                        nc.scalar.activation(
                            yt[:, s],
                            u[:, s],
                            mybir.ActivationFunctionType.Sign,
                            bias=neg_thresh[:],
                        )
                    nc.scalar.dma_start(y[t], yt[:])
                    u_prev = u
    nc.compile()
    return nc


# ---- legacy variant (previous session's kernel) for A/B ---------------------
def build_bass(free_w=HW, use_act=True, reps=1):
    import concourse.bacc as bacc
    import concourse.mybir as mybir
    from concourse.tile import TileContext

    f32 = mybir.dt.float32
    u8 = mybir.dt.uint8
    Alu = mybir.AluOpType

    nc = bacc.Bacc("TRN2", target_bir_lowering=False)
    x = nc.dram_tensor("x", [BPC, T, C, free_w], f32, kind="ExternalInput")
    y = nc.dram_tensor("y", [BPC, T, C, free_w], u8, kind="ExternalOutput")

    with TileContext(nc) as tc:
        with (
            tc.tile_pool(name="xp", bufs=6) as xp,
            tc.tile_pool(name="up", bufs=2) as up,
            tc.tile_pool(name="rp", bufs=2) as rp,
            tc.tile_pool(name="yp", bufs=3) as yp,
            tc.tile_pool(name="cp", bufs=1) as cp,
        ):
            neg_thresh = None
            if use_act:
                neg_thresh = cp.tile([C, 1], f32)
                nc.vector.memset(neg_thresh[:], -THRESH)
            for _rep in range(reps):
                r = None
                for t in range(T):
                    xt = xp.tile([C, BPC, free_w], f32)
                    nc.sync.dma_start(xt[:], x[:, t, :, :].rearrange("b c w -> c b w"))
                    if t == 0:
                        u = xt
                    else:
                        u = up.tile([C, BPC, free_w], f32)
                        nc.vector.scalar_tensor_tensor(
                            u[:], r[:], TAU, xt[:], Alu.mult, Alu.add
                        )
                    yt = yp.tile([C, BPC, free_w], u8)
                    if use_act:
                        nc.scalar.activation(
                            yt[:],
                            u[:],
                            mybir.ActivationFunctionType.Sign,
                            bias=neg_thresh[:],
                        )
                    else:
                        nc.vector.tensor_scalar(yt[:], u[:], THRESH, None, Alu.is_ge)
                    if t < T - 1:
                        rn = rp.tile([C, BPC, free_w], f32)
                        nc.vector.scalar_tensor_tensor(
                            rn[:], u[:], THRESH, u[:], Alu.is_lt, Alu.mult
                        )
                        r = rn
                    nc.scalar.dma_start(
                        y[:, t, :, :].rearrange("b c w -> c b w"), yt[:]
                    )
    nc.compile()
    return nc


def _get_nc():
    variant = os.environ.get("LIF_VARIANT", "fused")
    key = variant
    if key not in _nc_cache:
        if variant == "fused":
            spl = int(os.environ.get("LIF_SPL", "2"))
            nsplit = int(os.environ.get("LIF_NSPLIT", "2"))
            _nc_cache[key] = build_bass_fused(
                steps_per_load=spl, nsplit=nsplit
            )
        else:
            _nc_cache[key] = build_bass(HW, use_act=variant == "act")
    return _nc_cache[key]


def kernel(x):
    global LAST_RESULTS
    from concourse import bass_utils

    assert x.shape == (B, T, C, H, W) and x.dtype == np.float32
    variant = os.environ.get("LIF_VARIANT", "fused")
    nc = _get_nc()
    if variant == "fused":
        # per core i: x[4i:4i+4] as [T, C, BPC*HW] contiguous
        xs = np.ascontiguousarray(
            x.reshape(N_CORES, BPC, T, C, HW).transpose(0, 2, 3, 1, 4)
        ).reshape(N_CORES, T, C, FREE)
        in_maps = [{"x": xs[i]} for i in range(N_CORES)]
    else:
        xr = np.ascontiguousarray(x.reshape(B, T, C, HW))
        in_maps = [
            {"x": np.ascontiguousarray(xr[i * BPC : (i + 1) * BPC])}
            for i in range(N_CORES)
        ]
    res = bass_utils.run_bass_kernel_spmd(
        nc,
        in_maps,
        core_ids=list(range(N_CORES)),
        trace=bool(int(os.environ.get("LIF_TRACE", "0"))),
    )
    LAST_RESULTS = res
    out = np.empty((B, T, C, HW), dtype=np.float32)
    for i in range(N_CORES):
        yi = res.results[i]["y"]
        if variant == "fused":
            # yi [T, C, FREE]; Sign lands {255/0, 0, 1} in uint8; spike==1
            sp = yi.reshape(T, C, BPC, HW).transpose(2, 0, 1, 3)
            out[i * BPC : (i + 1) * BPC] = sp == 1
        else:
            out[i * BPC : (i + 1) * BPC] = yi == 1
    return out.reshape(B, T, C, H, W)
